# revision 4
# baseline (speedup 1.0000x reference)
"""Highway-LSTM Trainium2 kernel (data-parallel over batch on 8 NeuronCores).

Self-contained: hardcodes shapes B=32, T=512, D=H=512 from the problem spec.

Strategy per core (B_local = 4 sequences):
  Phase 1: input projection pi = x @ Wi.T + bi computed as a transposed GEMM
           (output-dim on partitions) into a DRAM scratch laid out so the scan
           can stream it back in t-blocks with wide contiguous runs.
  Phase 2: sequential scan over T. Everything lives in a transposed layout:
           partitions = hidden-dim chunk of 128, free dims = (k-chunk, batch).
           Recurrent matmul: psT[m] += WsT_chunk.T @ hT_chunk (stationary
           weights in bf16 => fast weight load), gates via ScalarE
           sigmoid/tanh, state hold via copy_predicated masking.
  Host does all layout transforms (transposes/permutes/unscrambles) in numpy.
"""

import os
import numpy as np
import ml_dtypes

P = 128
NCORES = 8
B, T, D, H = 32, 512, 512, 512
BL = B // NCORES            # 4 sequences per core
KH = H // P                 # 4 hidden chunks of 128
MI = 6 * H // P             # 24 input-projection m-chunks
MS = 5 * H // P             # 20 recurrent m-chunks
G = KH * BL                 # 16 = (k-chunk, batch) flat gate-tile width
TB = 32                     # scan t-block (y accumulation / pi promotion)

# gate order used on-device: (i, f, o, r, g, x6); reference order (i,f,g,o,r,x6)
PERM6 = [0, 1, 3, 4, 2, 5]
PERM5 = [0, 1, 3, 4, 2]

_CACHE = {}


def _build(T_steps, scan_bf16=True, in_f32r=True):
    import concourse.bacc as bacc
    import concourse.tile as tile
    import concourse.mybir as mybir
    import concourse.bass as bass

    f32 = mybir.dt.float32
    dt_scan = mybir.dt.bfloat16 if scan_bf16 else mybir.dt.float32r
    dt_in = mybir.dt.float32r if in_f32r else mybir.dt.float32
    AFT = mybir.ActivationFunctionType
    NTB = T_steps // TB

    nc = bacc.Bacc("TRN2", debug=False, num_devices=NCORES)

    xt = nc.dram_tensor("xt", [D, BL, T_steps], dt_in, kind="ExternalInput").ap()
    wiT = nc.dram_tensor("wiT", [D, 6 * H], dt_in, kind="ExternalInput").ap()
    wsT = nc.dram_tensor("wsT", [H, 5 * H], dt_scan, kind="ExternalInput").ap()
    pib = nc.dram_tensor("pib", [P, MI], f32, kind="ExternalInput").ap()
    maskd = nc.dram_tensor("maskd", [T_steps, G], mybir.dt.uint8,
                           kind="ExternalInput").ap()

    yt = nc.dram_tensor("yt", [NTB, P, TB * G], f32, kind="ExternalOutput").ap()
    ht = nc.dram_tensor("ht", [P, G], f32, kind="ExternalOutput").ap()
    ct = nc.dram_tensor("ct", [P, G], f32, kind="ExternalOutput").ap()

    with tile.TileContext(nc) as tc:
        with tc.tile_pool(name="const", bufs=1) as constp, \
             tc.tile_pool(name="dram", bufs=1, space="DRAM") as dramp:
            # ---- persistent state / constants ----
            ws_sb = []
            for k in range(KH):
                w = constp.tile([P, 5 * H], dt_scan, tag=f"ws{k}")
                nc.sync.dma_start(out=w, in_=wsT[k * P:(k + 1) * P, :])
                ws_sb.append(w)

            mask_sb = constp.tile([P, T_steps, G], mybir.dt.uint8, tag="mask")
            mask_bcast = bass.AP(
                tensor=maskd.tensor,
                offset=maskd.offset,
                ap=[[0, P]] + list(maskd.ap),
            )
            nc.gpsimd.dma_start(out=mask_sb, in_=mask_bcast)

            pib_sb = constp.tile([P, MI], f32, tag="pib")
            nc.sync.dma_start(out=pib_sb, in_=pib)

            h_sb = constp.tile([P, G], f32, tag="h")
            c_sb = constp.tile([P, G], f32, tag="c")
            h_bf = constp.tile([P, G], dt_scan, tag="hbf")
            nc.vector.memset(h_sb, 0.0)
            nc.vector.memset(c_sb, 0.0)
            nc.vector.memset(h_bf, 0.0)

            # scratch laid out so (m, b) merge into one AP dim for the scan
            pi_dram = dramp.tile([MI, BL, P, T_steps], f32, tag="pi")

            # ---- phase 1: pi^T = WiT_chunk.T @ x^T, + bias, to DRAM scratch ----
            with tc.tile_pool(name="ph1", bufs=1) as ph1, \
                 tc.tile_pool(name="ph1ps", bufs=2, space="PSUM") as ph1ps, \
                 tc.tile_pool(name="ph1ev", bufs=4) as ph1ev:
                wi_sb = []
                xt_sb = []
                for k in range(KH):
                    w = ph1.tile([P, 6 * H], dt_in, tag=f"wi{k}")
                    nc.sync.dma_start(out=w, in_=wiT[k * P:(k + 1) * P, :])
                    wi_sb.append(w)
                    x_ = ph1.tile([P, BL, T_steps], dt_in, tag=f"xt{k}")
                    nc.sync.dma_start(out=x_, in_=xt[k * P:(k + 1) * P, :, :])
                    xt_sb.append(x_)

                for m in range(MI):
                    pss = [ph1ps.tile([P, T_steps], f32, tag=f"ps{b}",
                                      name=f"pss{b}")
                           for b in range(BL)]
                    for k in range(KH):
                        lhsT = wi_sb[k][:, m * P:(m + 1) * P]
                        for b in range(BL):
                            nc.tensor.matmul(
                                pss[b], lhsT, xt_sb[k][:, b, :],
                                start=(k == 0), stop=(k == KH - 1),
                            )
                    for b in range(BL):
                        ev = ph1ev.tile([P, T_steps], f32, tag="ev")
                        nc.scalar.activation(
                            ev, pss[b], AFT.Identity, bias=pib_sb[:, m:m + 1],
                        )
                        nc.sync.dma_start(out=pi_dram[m, b, :, :], in_=ev)

            # ---- phase 2: the scan ----
            with tc.tile_pool(name="pp", bufs=2) as pipool, \
                 tc.tile_pool(name="yp", bufs=2) as ypool, \
                 tc.tile_pool(name="sps", bufs=2, space="PSUM") as pspool, \
                 tc.tile_pool(name="ew", bufs=2) as ewp:
                for tb in range(NTB):
                    pp = pipool.tile([P, MI * BL, TB], f32, tag="pp")
                    src = pi_dram[:, :, :, tb * TB:(tb + 1) * TB]
                    nc.sync.dma_start(
                        out=pp, in_=src.rearrange("m b p t -> p (m b) t"))

                    y_t = ypool.tile([P, TB, G], f32, tag="y")
                    nc.gpsimd.memset(y_t, 0.0)

                    for tt in range(TB):
                        t = tb * TB + tt
                        ps = pspool.tile([P, MS * BL], f32, tag="ps")
                        for m in range(MS):
                            for k in range(KH):
                                nc.tensor.matmul(
                                    ps[:, m * BL:(m + 1) * BL],
                                    ws_sb[k][:, m * P:(m + 1) * P],
                                    h_bf[:, k * BL:(k + 1) * BL],
                                    start=(k == 0), stop=(k == KH - 1),
                                )
                        gs = ewp.tile([P, MS * BL], f32, tag="gs")
                        nc.vector.tensor_add(gs, ps, pp[:, 0:MS * BL, tt])
                        # cols: g*16..g*16+15 per gate, gate order (i,f,o,r,g)
                        sg = ewp.tile([P, 4 * G], f32, tag="sg")
                        nc.scalar.activation(sg, gs[:, 0:4 * G], AFT.Sigmoid)
                        tg = ewp.tile([P, G], f32, tag="tg")
                        nc.scalar.activation(tg, gs[:, 4 * G:5 * G], AFT.Tanh)

                        pi6 = pp[:, 5 * G:6 * G, tt]
                        ig = ewp.tile([P, G], f32, tag="ig")
                        nc.vector.tensor_mul(ig, sg[:, 0:G], tg)
                        fc = ewp.tile([P, G], f32, tag="fc")
                        nc.vector.tensor_mul(fc, sg[:, G:2 * G], c_sb)
                        cn = ewp.tile([P, G], f32, tag="cn")
                        nc.vector.tensor_add(cn, ig, fc)
                        th = ewp.tile([P, G], f32, tag="th")
                        nc.scalar.activation(th, cn, AFT.Tanh)
                        ot = ewp.tile([P, G], f32, tag="ot")
                        nc.vector.tensor_mul(ot, sg[:, 2 * G:3 * G], th)
                        w1 = ewp.tile([P, G], f32, tag="w1")
                        nc.vector.tensor_sub(w1, ot, pi6)
                        w2 = ewp.tile([P, G], f32, tag="w2")
                        nc.vector.tensor_mul(w2, sg[:, 3 * G:4 * G], w1)
                        out = ewp.tile([P, G], f32, tag="out")
                        nc.vector.tensor_add(out, w2, pi6)

                        mt = mask_sb[:, t, :]
                        nc.vector.copy_predicated(y_t[:, tt, :], mt, out)
                        nc.vector.copy_predicated(h_sb, mt, out)
                        nc.vector.copy_predicated(c_sb, mt, cn)
                        nc.vector.tensor_copy(h_bf, h_sb)

                    nc.sync.dma_start(out=yt[tb, :, :], in_=y_t)

            nc.sync.dma_start(out=ht, in_=h_sb)
            nc.sync.dma_start(out=ct, in_=c_sb)

    nc.compile()
    return nc


def _get(T_steps, scan_bf16=True, in_f32r=True):
    key = (T_steps, scan_bf16, in_f32r)
    if key not in _CACHE:
        _CACHE[key] = _build(T_steps, scan_bf16, in_f32r)
    return _CACHE[key]


def _prep_in_maps(x, lengths, Wi, bi, Ws, bs, T_steps, scan_bf16=True):
    # permute gate blocks to device order
    wiT = np.ascontiguousarray(
        Wi.reshape(6, H, D)[PERM6].reshape(6 * H, D).T).astype(np.float32)
    ws_p = Ws.reshape(5, H, H)[PERM5].reshape(5 * H, H)
    wsT = np.ascontiguousarray(ws_p.T)
    wsT = wsT.astype(ml_dtypes.bfloat16) if scan_bf16 else wsT.astype(np.float32)
    bi_p = bi.reshape(6, H)[PERM6].reshape(6 * H).astype(np.float32)
    bs_p = bs.reshape(5, H)[PERM5].reshape(5 * H).astype(np.float32)
    vec24 = bi_p.reshape(MI, P).copy()
    vec24[:MS] += bs_p.reshape(MS, P)
    pib = np.ascontiguousarray(vec24.T)  # [P, MI]

    in_maps = []
    for c in range(NCORES):
        xs = x[c * BL:(c + 1) * BL, :T_steps, :]      # [BL, T, D]
        xt = np.ascontiguousarray(xs.transpose(2, 0, 1)).astype(np.float32)
        ls = lengths[c * BL:(c + 1) * BL]
        m = (np.arange(T_steps)[:, None] < ls[None, :]).astype(np.uint8)
        maskd = np.ascontiguousarray(
            np.broadcast_to(m[:, None, :], (T_steps, KH, BL))
        ).reshape(T_steps, G)
        in_maps.append({
            "xt": xt, "wiT": wiT, "wsT": wsT, "pib": pib, "maskd": maskd,
        })
    return in_maps


def _unscramble(results, T_steps):
    NTB = T_steps // TB
    y = np.empty((B, T_steps, H), np.float32)
    hT = np.empty((B, H), np.float32)
    cT = np.empty((B, H), np.float32)
    for c in range(NCORES):
        r = results[c]
        ya = r["yt"].reshape(NTB, P, TB, KH, BL)
        # y[b, tbk*TB+tt, k*128+p] = ya[tbk, p, tt, k, b]
        y[c * BL:(c + 1) * BL] = (
            ya.transpose(4, 0, 2, 3, 1).reshape(BL, T_steps, H))
        hT[c * BL:(c + 1) * BL] = (
            r["ht"].reshape(P, KH, BL).transpose(2, 1, 0).reshape(BL, H))
        cT[c * BL:(c + 1) * BL] = (
            r["ct"].reshape(P, KH, BL).transpose(2, 1, 0).reshape(BL, H))
    return y, hT, cT


def run(x, lengths, Wi, bi, Ws, bs, T_steps=T, scan_bf16=True, in_f32r=True,
        trace=False):
    from concourse import bass_utils
    nc = _get(T_steps, scan_bf16, in_f32r)
    in_maps = _prep_in_maps(x, lengths, Wi, bi, Ws, bs, T_steps, scan_bf16)
    res = bass_utils.run_bass_kernel_spmd(
        nc, in_maps, core_ids=list(range(NCORES)), trace=trace)
    outs = _unscramble(res.results, T_steps)
    return outs, res


def kernel(x, lengths, Wi, bi, Ws, bs):
    (y, hT, cT), _ = run(
        np.asarray(x), np.asarray(lengths), np.asarray(Wi),
        np.asarray(bi), np.asarray(Ws), np.asarray(bs))
    return y, hT, cT


# revision 6
# speedup vs baseline: 2.7976x; 2.7976x over previous
"""Highway-LSTM Trainium2 kernel (data-parallel over batch on 8 NeuronCores).

Self-contained: hardcodes shapes B=32, T=512, D=H=512 from the problem spec.

Strategy per core (B_local = 4 sequences):
  Phase 1: input projection pi = x @ Wi.T + bi computed as a transposed GEMM
           (output-dim on partitions) into a DRAM scratch laid out so the scan
           can stream it back in t-blocks with wide contiguous runs.
  Phase 2: sequential scan over T. Everything lives in a transposed layout:
           partitions = hidden-dim chunk of 128, free dims = (k-chunk, batch).
           Recurrent matmul: psT[m] += WsT_chunk.T @ hT_chunk (stationary
           weights in bf16 => fast weight load), gates via ScalarE
           sigmoid/tanh, state hold via copy_predicated masking.
  Host does all layout transforms (transposes/permutes/unscrambles) in numpy.
"""

import os
import numpy as np
import ml_dtypes

P = 128
NCORES = 8
B, T, D, H = 32, 512, 512, 512
BL = B // NCORES            # 4 sequences per core
KH = H // P                 # 4 hidden chunks of 128
MI = 6 * H // P             # 24 input-projection m-chunks
MS = 5 * H // P             # 20 recurrent m-chunks
G = KH * BL                 # 16 = (k-chunk, batch) flat gate-tile width
TB = 32                     # scan t-block (y accumulation / pi promotion)

# gate order used on-device: (i, f, o, r, g, x6); reference order (i,f,g,o,r,x6)
PERM6 = [0, 1, 3, 4, 2, 5]
PERM5 = [0, 1, 3, 4, 2]

_CACHE = {}


def _build(T_steps, scan_bf16=True, in_f32r=True, mode="full", pimm=True):
    import concourse.bacc as bacc
    import concourse.tile as tile
    import concourse.mybir as mybir
    import concourse.bass as bass

    f32 = mybir.dt.float32
    dt_scan = mybir.dt.bfloat16 if scan_bf16 else mybir.dt.float32r
    dt_in = mybir.dt.float32r if in_f32r else mybir.dt.float32
    AFT = mybir.ActivationFunctionType
    NTB = T_steps // TB

    nc = bacc.Bacc("TRN2", debug=False, num_devices=NCORES)

    xt = nc.dram_tensor("xt", [D, BL, T_steps], dt_in, kind="ExternalInput").ap()
    wiT = nc.dram_tensor("wiT", [D, 6 * H], dt_in, kind="ExternalInput").ap()
    wsT = nc.dram_tensor("wsT", [H, 5 * H], dt_scan, kind="ExternalInput").ap()
    pib = nc.dram_tensor("pib", [P, MI], f32, kind="ExternalInput").ap()
    identd = nc.dram_tensor("identd", [P, P], f32, kind="ExternalInput").ap()
    maskd = nc.dram_tensor("maskd", [T_steps, G], mybir.dt.uint8,
                           kind="ExternalInput").ap()

    yt = nc.dram_tensor("yt", [NTB, P, TB * G], f32, kind="ExternalOutput").ap()
    ht = nc.dram_tensor("ht", [P, G], f32, kind="ExternalOutput").ap()
    ct = nc.dram_tensor("ct", [P, G], f32, kind="ExternalOutput").ap()

    with tile.TileContext(nc) as tc:
        with tc.tile_pool(name="const", bufs=1) as constp, \
             tc.tile_pool(name="dram", bufs=1, space="DRAM") as dramp:
            # ---- persistent state / constants ----
            ws_sb = []
            for k in range(KH):
                w = constp.tile([P, 5 * H], dt_scan, tag=f"ws{k}")
                nc.sync.dma_start(out=w, in_=wsT[k * P:(k + 1) * P, :])
                ws_sb.append(w)

            mask_sb = constp.tile([P, T_steps, G], mybir.dt.uint8, tag="mask")
            mask_bcast = bass.AP(
                tensor=maskd.tensor,
                offset=maskd.offset,
                ap=[[0, P]] + list(maskd.ap),
            )
            nc.gpsimd.dma_start(out=mask_sb, in_=mask_bcast)

            pib_sb = constp.tile([P, MI], f32, tag="pib")
            nc.sync.dma_start(out=pib_sb, in_=pib)

            ident = constp.tile([P, P], f32, tag="ident")
            nc.sync.dma_start(out=ident, in_=identd)

            h_sb = constp.tile([P, G], f32, tag="h")
            c_sb = constp.tile([P, G], f32, tag="c")
            h_bf = constp.tile([P, G], dt_scan, tag="hbf")
            nc.vector.memset(h_sb, 0.0)
            nc.vector.memset(c_sb, 0.0)
            nc.vector.memset(h_bf, 0.0)

            # scratch laid out so (m, b) merge into one AP dim for the scan
            pi_dram = dramp.tile([MI, BL, P, T_steps], f32, tag="pi")

            # ---- phase 1: pi^T = WiT_chunk.T @ x^T, + bias, to DRAM scratch ----
            with tc.tile_pool(name="ph1", bufs=1) as ph1, \
                 tc.tile_pool(name="ph1ps", bufs=2, space="PSUM") as ph1ps, \
                 tc.tile_pool(name="ph1ev", bufs=4) as ph1ev:
                wi_sb = []
                xt_sb = []
                for k in range(KH):
                    w = ph1.tile([P, 6 * H], dt_in, tag=f"wi{k}")
                    nc.sync.dma_start(out=w, in_=wiT[k * P:(k + 1) * P, :])
                    wi_sb.append(w)
                    x_ = ph1.tile([P, BL, T_steps], dt_in, tag=f"xt{k}")
                    nc.sync.dma_start(out=x_, in_=xt[k * P:(k + 1) * P, :, :])
                    xt_sb.append(x_)

                for m in range(MI):
                    pss = [ph1ps.tile([P, T_steps], f32, tag=f"ps{b}",
                                      name=f"pss{b}")
                           for b in range(BL)]
                    for k in range(KH):
                        lhsT = wi_sb[k][:, m * P:(m + 1) * P]
                        for b in range(BL):
                            nc.tensor.matmul(
                                pss[b], lhsT, xt_sb[k][:, b, :],
                                start=(k == 0), stop=(k == KH - 1),
                            )
                    for b in range(BL):
                        ev = ph1ev.tile([P, T_steps], f32, tag="ev")
                        nc.scalar.activation(
                            ev, pss[b], AFT.Identity, bias=pib_sb[:, m:m + 1],
                        )
                        nc.sync.dma_start(out=pi_dram[m, b, :, :], in_=ev)

            # ---- phase 2: the scan ----
            with tc.tile_pool(name="pp", bufs=2) as pipool, \
                 tc.tile_pool(name="yp", bufs=2) as ypool, \
                 tc.tile_pool(name="sps", bufs=2, space="PSUM") as pspool, \
                 tc.tile_pool(name="ew", bufs=2) as ewp:
                for tb in range(NTB):
                    pp = pipool.tile([P, MI * BL, TB], f32, tag="pp")
                    src = pi_dram[:, :, :, tb * TB:(tb + 1) * TB]
                    nc.sync.dma_start(
                        out=pp, in_=src.rearrange("m b p t -> p (m b) t"))

                    y_t = ypool.tile([P, TB, G], f32, tag="y")
                    nc.gpsimd.memset(y_t, 0.0)

                    for tt in range(TB):
                        t = tb * TB + tt
                        ps = pspool.tile([P, MS * BL], f32, tag="ps")
                        if pimm:
                            # accumulate pi into PSUM with one identity matmul
                            nc.tensor.matmul(
                                ps, ident, pp[:, 0:MS * BL, tt],
                                start=True, stop=False, skip_group_check=True)
                        if mode != "ew":
                            for m in range(MS):
                                for k in range(KH):
                                    nc.tensor.matmul(
                                        ps[:, m * BL:(m + 1) * BL],
                                        ws_sb[k][:, m * P:(m + 1) * P],
                                        h_bf[:, k * BL:(k + 1) * BL],
                                        start=(k == 0) and not pimm,
                                        stop=(k == KH - 1),
                                        skip_group_check=True,
                                    )
                        if mode == "pe":
                            continue
                        if pimm:
                            gsrc = ps
                        else:
                            gs = ewp.tile([P, MS * BL], f32, tag="gs")
                            nc.vector.tensor_add(gs, ps, pp[:, 0:MS * BL, tt])
                            gsrc = gs
                        # cols: g*16..g*16+15 per gate, gate order (i,f,o,r,g)
                        sg = ewp.tile([P, 4 * G], f32, tag="sg")
                        nc.scalar.activation(sg, gsrc[:, 0:4 * G], AFT.Sigmoid)
                        tg = ewp.tile([P, G], f32, tag="tg")
                        nc.scalar.activation(tg, gsrc[:, 4 * G:5 * G], AFT.Tanh)

                        pi6 = pp[:, 5 * G:6 * G, tt]
                        ig = ewp.tile([P, G], f32, tag="ig")
                        nc.vector.tensor_mul(ig, sg[:, 0:G], tg)
                        fc = ewp.tile([P, G], f32, tag="fc")
                        nc.vector.tensor_mul(fc, sg[:, G:2 * G], c_sb)
                        cn = ewp.tile([P, G], f32, tag="cn")
                        nc.vector.tensor_add(cn, ig, fc)
                        th = ewp.tile([P, G], f32, tag="th")
                        nc.scalar.activation(th, cn, AFT.Tanh)
                        ot = ewp.tile([P, G], f32, tag="ot")
                        nc.vector.tensor_mul(ot, sg[:, 2 * G:3 * G], th)
                        w1 = ewp.tile([P, G], f32, tag="w1")
                        nc.vector.tensor_sub(w1, ot, pi6)
                        w2 = ewp.tile([P, G], f32, tag="w2")
                        nc.vector.tensor_mul(w2, sg[:, 3 * G:4 * G], w1)
                        out = ewp.tile([P, G], f32, tag="out")
                        nc.vector.tensor_add(out, w2, pi6)

                        mt = mask_sb[:, t, :]
                        ob = ewp.tile([P, G], dt_scan, tag="ob")
                        nc.vector.tensor_copy(ob, out)
                        nc.vector.copy_predicated(h_bf, mt, ob)
                        nc.vector.copy_predicated(y_t[:, tt, :], mt, out)
                        nc.vector.copy_predicated(h_sb, mt, out)
                        nc.vector.copy_predicated(c_sb, mt, cn)

                    nc.sync.dma_start(out=yt[tb, :, :], in_=y_t)

            nc.sync.dma_start(out=ht, in_=h_sb)
            nc.sync.dma_start(out=ct, in_=c_sb)

    nc.compile()
    return nc


def _get(T_steps, scan_bf16=True, in_f32r=True, mode="full", pimm=True):
    key = (T_steps, scan_bf16, in_f32r, mode, pimm)
    if key not in _CACHE:
        _CACHE[key] = _build(T_steps, scan_bf16, in_f32r, mode, pimm)
    return _CACHE[key]


def _prep_in_maps(x, lengths, Wi, bi, Ws, bs, T_steps, scan_bf16=True):
    # permute gate blocks to device order
    wiT = np.ascontiguousarray(
        Wi.reshape(6, H, D)[PERM6].reshape(6 * H, D).T).astype(np.float32)
    ws_p = Ws.reshape(5, H, H)[PERM5].reshape(5 * H, H)
    wsT = np.ascontiguousarray(ws_p.T)
    wsT = wsT.astype(ml_dtypes.bfloat16) if scan_bf16 else wsT.astype(np.float32)
    bi_p = bi.reshape(6, H)[PERM6].reshape(6 * H).astype(np.float32)
    bs_p = bs.reshape(5, H)[PERM5].reshape(5 * H).astype(np.float32)
    vec24 = bi_p.reshape(MI, P).copy()
    vec24[:MS] += bs_p.reshape(MS, P)
    pib = np.ascontiguousarray(vec24.T)  # [P, MI]

    in_maps = []
    for c in range(NCORES):
        xs = x[c * BL:(c + 1) * BL, :T_steps, :]      # [BL, T, D]
        xt = np.ascontiguousarray(xs.transpose(2, 0, 1)).astype(np.float32)
        ls = lengths[c * BL:(c + 1) * BL]
        m = (np.arange(T_steps)[:, None] < ls[None, :]).astype(np.uint8)
        maskd = np.ascontiguousarray(
            np.broadcast_to(m[:, None, :], (T_steps, KH, BL))
        ).reshape(T_steps, G)
        in_maps.append({
            "xt": xt, "wiT": wiT, "wsT": wsT, "pib": pib, "maskd": maskd,
            "identd": np.eye(P, dtype=np.float32),
        })
    return in_maps


def _unscramble(results, T_steps):
    NTB = T_steps // TB
    y = np.empty((B, T_steps, H), np.float32)
    hT = np.empty((B, H), np.float32)
    cT = np.empty((B, H), np.float32)
    for c in range(NCORES):
        r = results[c]
        ya = r["yt"].reshape(NTB, P, TB, KH, BL)
        # y[b, tbk*TB+tt, k*128+p] = ya[tbk, p, tt, k, b]
        y[c * BL:(c + 1) * BL] = (
            ya.transpose(4, 0, 2, 3, 1).reshape(BL, T_steps, H))
        hT[c * BL:(c + 1) * BL] = (
            r["ht"].reshape(P, KH, BL).transpose(2, 1, 0).reshape(BL, H))
        cT[c * BL:(c + 1) * BL] = (
            r["ct"].reshape(P, KH, BL).transpose(2, 1, 0).reshape(BL, H))
    return y, hT, cT


def run(x, lengths, Wi, bi, Ws, bs, T_steps=T, scan_bf16=True, in_f32r=True,
        trace=False, mode="full", pimm=True):
    from concourse import bass_utils
    nc = _get(T_steps, scan_bf16, in_f32r, mode, pimm)
    in_maps = _prep_in_maps(x, lengths, Wi, bi, Ws, bs, T_steps, scan_bf16)
    res = bass_utils.run_bass_kernel_spmd(
        nc, in_maps, core_ids=list(range(NCORES)), trace=trace)
    outs = _unscramble(res.results, T_steps)
    return outs, res


def kernel(x, lengths, Wi, bi, Ws, bs):
    (y, hT, cT), _ = run(
        np.asarray(x), np.asarray(lengths), np.asarray(Wi),
        np.asarray(bi), np.asarray(Ws), np.asarray(bs))
    return y, hT, cT
